# revision 36
# baseline (speedup 1.0000x reference)
"""Davies-Bouldin loss kernel for 8 TRN2 NeuronCores (Bass/Tile).

Class-sorted data-parallel design. Host sorts samples by class and assigns
classes [8k, 8k+8) to core k (static map). Each 128-partition subtile holds
127 samples of ONE class plus that class's centroid row baked at partition
127. x rows are pre-scaled by 1/count[class] during the host bf16 cast.

Per subtile on device:
  diff = xs - c     one matmul with CONSTANT weights [I127 ; -ones(127)]
                    (the -1 row multiplies the baked centroid row)
  v2   = sum(diff^2) rowwise: DVE tensor_tensor_reduce or ACT Square+accum
                    (subtiles split across both engines to balance)
  vec  = sqrt(v2)   batched per macro on ACT, written to blk col 256
  pacc[8,257] += oa^T @ [xs | vec]   scatter matmul (oa onehot host-baked
                    in blk cols 257:265)

AllGather of per-core [8,257] partials -> [64,257]; identical scalar tail
on every core.
"""

import numpy as np
import ml_dtypes

import concourse.bass as bass
import concourse.mybir as mybir
from concourse.bass_utils import run_bass_kernel_spmd
from concourse.tile import TileContext

C = 64            # classes
D = 256           # feature dim
NCORES = 8
CL = C // NCORES  # classes per core
SS = 127          # samples per subtile (partition 127 = centroid row)
A = 8             # subtiles per macro
NSUB = 280        # subtiles per core (padded, fixed at build)
NM = NSUB // A    # macros per core
W = 272           # blk row width in bf16 elements (544B rows)
SC = 257          # scatter width: xs(256) | vec(1)
SPLIT = 200       # subtiles in the first collective chunk
OAC = 257         # oa onehot columns start
F32 = mybir.dt.float32
BF16 = mybir.dt.bfloat16

AF = mybir.ActivationFunctionType
OP = mybir.AluOpType

# which subtiles of each macro reduce on ACT (rest go to DVE).
# ACT gets 4 of 8 every 5th macro, else 3 -> DVE averages 4.8/macro.
def _act_subtiles(m):
    return (4, 5, 6, 7) if m % 5 == 0 else (5, 6, 7)


def _split_excess_waits(nc, max_waits=1):
    """This walrus build only accepts one sync-wait per instruction;
    hoist excess waits onto prepended NoOps on the same engine."""
    k = 0
    for f in nc.m.functions:
        for b in f.blocks:
            insts = b.instructions
            if not any(
                i.sync_info and i.sync_info.on_wait and len(i.sync_info.on_wait) > max_waits
                for i in insts
            ):
                continue
            out = []
            for inst in insts:
                si = inst.sync_info
                if si and si.on_wait and len(si.on_wait) > max_waits:
                    waits = list(si.on_wait)
                    extra, keep = waits[:-max_waits], waits[-max_waits:]
                    for j in range(0, len(extra), max_waits):
                        chunk = extra[j:j + max_waits]
                        nop = mybir.InstNoOp(name=f"I-splitw-{k}", ins=[], outs=[])
                        k += 1
                        nop.engine = inst.engine
                        nop.sync_info = mybir.SyncInfo(on_wait=chunk, on_update=[])
                        try:
                            nc.register_instruction(nop, overwrite=True)
                        except Exception:
                            pass
                        out.append(nop)
                    inst.sync_info = mybir.SyncInfo(
                        on_wait=keep, on_update=list(si.on_update or [])
                    )
                out.append(inst)
            b.instructions = out
    return k


def build_module():
    nc = bass.Bass("TRN2", target_bir_lowering=False, debug=False, num_devices=NCORES)

    blkp = nc.declare_dram_parameter("blk", [NSUB * 128, W], BF16, isOutput=False)
    augsp = nc.declare_dram_parameter("augs", [128, 2 * SS], BF16, isOutput=False)
    cnsqp = nc.declare_dram_parameter("cnsq", [128, NSUB], F32, isOutput=False)
    catcp = nc.declare_dram_parameter("catc", [C, 452], F32, isOutput=False)
    onesr = nc.declare_dram_parameter("onesr", [1, C], F32, isOutput=False)
    outp = nc.declare_dram_parameter("out", [1, 1], F32, isOutput=True)

    cc_in0 = nc.dram_tensor("cc_in0", [CL, SC], F32)
    cc_out0 = nc.dram_tensor("cc_out0", [C, SC], F32, addr_space="Shared")
    cc_in1 = nc.dram_tensor("cc_in1", [CL, SC], F32)
    cc_out1 = nc.dram_tensor("cc_out1", [C, SC], F32, addr_space="Shared")

    cc_sem = nc.alloc_semaphore("cc_sem")
    ccd_sem = nc.alloc_semaphore("ccd_sem")

    with TileContext(nc) as tc:
        with (
            tc.tile_pool(name="consts", bufs=1) as cpool,
            tc.tile_pool(name="xin", bufs=6) as xpool,
            tc.tile_pool(name="smalls", bufs=3) as spool,
            tc.tile_pool(name="ttscr", bufs=2) as tscpool,
            tc.tile_pool(name="sqscr", bufs=2) as sqpool,
            tc.tile_pool(name="psdiff", bufs=5, space="PSUM") as pdpool,
            tc.tile_pool(name="psacc", bufs=1, space="PSUM") as papool,
            tc.tile_pool(name="pstail", bufs=1, space="PSUM") as ptpool,
            tc.tile_pool(name="tail", bufs=1) as tpool,
        ):
            # ---- constant loads (packed, spread across engine queues) ----
            sb_augs = cpool.tile([128, 2 * SS], BF16, tag="augs")
            nc.scalar.dma_start(out=sb_augs[:], in_=augsp[:])
            sb_aug = sb_augs[:, 0:SS]
            sb_aug2 = sb_augs[:, SS:2 * SS]
            sb_cnsq = cpool.tile([128, NSUB], F32, tag="cnsq")
            nc.gpsimd.dma_start(out=sb_cnsq[:], in_=cnsqp[:])
            sb_catc = cpool.tile([C, 452], F32, tag="catc")
            nc.scalar.dma_start(out=sb_catc[:], in_=catcp[:])
            sb_wsc = sb_catc[:, 0:64]
            sb_eyebig = sb_catc[:, 64:128]
            sb_iden = sb_catc[:, 128:192]
            sb_cent = sb_catc[:, 192:448]
            sb_dist = sb_catc[:, 448:449]
            sb_ic = sb_catc[:, 449:450]
            sb_ones = sb_catc[:, 450:451]
            sb_onesr = cpool.tile([1, C], F32, tag="onesr")
            nc.scalar.dma_start(out=sb_onesr[:], in_=onesr[:])

            pacc0_t = papool.tile([CL, SC], F32, tag="pacc0")
            pacc1_t = papool.tile([CL, SC], F32, tag="pacc1")
            pacc0 = pacc0_t[:]
            pacc1 = pacc1_t[:]

            # ---- main loop; scatters trail one macro for PE pipelining ----
            prev = None
            for m in range(NM):
                blk = xpool.tile([128, A, W], BF16, tag="blk")
                src = blkp[m * A * 128:(m + 1) * A * 128, :].rearrange(
                    "(p a) d -> p a d", p=128
                )
                nc.sync.dma_start(out=blk[:], in_=src)

                act_set = _act_subtiles(m)
                diffs = []
                for h in range(A // 2):
                    # two subtiles share one PSUM bank to stretch the pool
                    df2 = pdpool.tile([SS, 2, D], F32, tag="df")
                    for i in range(2):
                        a = 2 * h + i
                        # ACT path: pg = xs - c, v2 = sum(pg^2)
                        # DVE path: pg = xs - 2c, v2 = cn2 + sum(xs * pg)
                        nc.tensor.matmul(
                            df2[:, i, :],
                            lhsT=(sb_aug if a in act_set else sb_aug2)[:],
                            rhs=blk[:, a, 0:D],
                            start=True,
                            stop=True,
                        )
                    diffs.append(df2)

                v2all = spool.tile([SS, A], F32, tag="v2all")
                for a in range(A):
                    df = diffs[a // 2][:, a % 2, :]
                    if a in act_set:
                        sq_scr = sqpool.tile([SS, D], BF16, tag="sq_scr")
                        nc.scalar.activation(
                            out=sq_scr[:], in_=df, func=AF.Square,
                            accum_out=v2all[:, a:a + 1],
                        )
                    else:
                        tt_scr = tscpool.tile([SS, 1], BF16, tag="tt_scr")
                        nc.vector.scalar_tensor_tensor(
                            out=tt_scr.broadcast_to((SS, D)),
                            in0=blk[0:SS, a, 0:D], scalar=1.0, in1=df,
                            op0=OP.bypass, op1=OP.mult,
                            accum_out=v2all[:, a:a + 1],
                        )
                # add cn2 for the DVE-dot subtiles (host zeroes ACT columns)
                v2f = spool.tile([SS, A], F32, tag="v2f")
                nc.vector.tensor_tensor(
                    out=v2f[:], in0=v2all[:],
                    in1=sb_cnsq[0:SS, m * A:(m + 1) * A], op=OP.add,
                )
                nc.scalar.activation(
                    out=blk[0:SS, :, D], in_=v2f[:], func=AF.Sqrt,
                )

                if prev is not None:
                    pm, pblk = prev
                    for a in range(A):
                        g = pm * A + a
                        pa = pacc0 if g < SPLIT else pacc1
                        nc.tensor.matmul(
                            pa,
                            lhsT=pblk[0:SS, a, OAC:OAC + CL],
                            rhs=pblk[0:SS, a, 0:SC],
                            start=(g == 0 or g == SPLIT),
                            stop=(g == SPLIT - 1),
                        )
                    if pm == SPLIT // A - 1:
                        # pacc0 complete: overlap its all-gather with the
                        # rest of the main loop (scalar queue keeps sync free)
                        acc0_sb = tpool.tile([CL, SC], F32, tag="acc0_sb")
                        nc.vector.tensor_scalar(
                            out=acc0_sb[:], in0=pacc0, scalar1=1.0,
                            scalar2=None, op0=OP.mult,
                        )
                        with tc.tile_critical():
                            nc.scalar.dma_start(
                                out=cc_in0[:], in_=acc0_sb[:]
                            ).then_inc(ccd_sem, 16)
                            nc.gpsimd.wait_ge(ccd_sem, 16)
                            nc.gpsimd.collective_compute(
                                "AllGather",
                                OP.bypass,
                                replica_groups=[list(range(NCORES))],
                                ins=[cc_in0[:]],
                                outs=[cc_out0[:]],
                            ).then_inc(cc_sem, 1)
                prev = (m, blk)

            pm, pblk = prev
            for a in range(A):
                g = pm * A + a
                nc.tensor.matmul(
                    pacc1,
                    lhsT=pblk[0:SS, a, OAC:OAC + CL],
                    rhs=pblk[0:SS, a, 0:SC],
                    start=False,
                    stop=(a == A - 1),
                )

            # ---- second all-gather for the last chunk ----
            acc1_sb = tpool.tile([CL, SC], F32, tag="acc1_sb")
            nc.vector.tensor_scalar(
                out=acc1_sb[:], in0=pacc1, scalar1=1.0, scalar2=None,
                op0=OP.mult,
            )
            g0_sb = tpool.tile([C, SC], F32, tag="g0_sb")
            g1_sb = tpool.tile([C, SC], F32, tag="g1_sb")
            with tc.tile_critical():
                nc.scalar.dma_start(out=cc_in1[:], in_=acc1_sb[:]).then_inc(ccd_sem, 16)
                nc.gpsimd.wait_ge(ccd_sem, 32)
                nc.gpsimd.collective_compute(
                    "AllGather",
                    OP.bypass,
                    replica_groups=[list(range(NCORES))],
                    ins=[cc_in1[:]],
                    outs=[cc_out1[:]],
                ).then_inc(cc_sem, 1)
                nc.scalar.wait_ge(cc_sem, 2)
                nc.scalar.dma_start(out=g0_sb[:], in_=cc_out0[:]).then_inc(ccd_sem, 16)
                nc.scalar.dma_start(out=g1_sb[:], in_=cc_out1[:]).then_inc(ccd_sem, 16)
                nc.sync.wait_ge(ccd_sem, 64)
            allsum = tpool.tile([C, SC], F32, tag="allsum")
            nc.vector.tensor_tensor(
                out=allsum[:], in0=g0_sb[:], in1=g1_sb[:], op=OP.add
            )

            # ---- scalar loss tail (identical on every core) ----
            cn = tpool.tile([C, D], F32, tag="cn")
            nc.vector.tensor_tensor(
                out=cn[:], in0=allsum[:, 0:D], in1=sb_cent[:], op=OP.add
            )
            sq = tpool.tile([C, 1], F32, tag="sq")
            sq_scr2 = tpool.tile([C, 1], BF16, tag="sq_scr2")
            nc.vector.scalar_tensor_tensor(
                out=sq_scr2.broadcast_to((C, D)), in0=cn[:], scalar=1.0,
                in1=cn[:], op0=OP.bypass, op1=OP.mult,
                accum_out=sq[:],
            )
            absr = tpool.tile([C, 1], F32, tag="absr")
            nc.vector.tensor_reduce(
                out=absr[:], in_=cn[:], axis=mybir.AxisListType.X, op=OP.add,
                apply_absolute_value=True,
            )
            # s = sqrt(dist + sum_vec) * ic
            svp = tpool.tile([C, 1], F32, tag="svp")
            nc.vector.tensor_tensor(
                out=svp[:], in0=allsum[:, D:D + 1], in1=sb_dist[:], op=OP.add
            )
            sroot = tpool.tile([C, 1], F32, tag="sroot")
            nc.scalar.activation(out=sroot[:], in_=svp[:], func=AF.Sqrt)
            s_sb = tpool.tile([C, 1], F32, tag="s_sb")
            nc.vector.tensor_scalar(
                out=s_sb[:], in0=sroot[:], scalar1=sb_ic[:], scalar2=None,
                op0=OP.mult,
            )
            # cn^T (two 128-wide chunks) for CN = cn @ cn^T
            cnt_sb = tpool.tile([128, 128], F32, tag="cnt_sb")
            for h in range(2):
                pt = ptpool.tile([128, C], F32, tag="pt")
                nc.tensor.transpose(
                    pt[:], in_=cn[:, h * 128:(h + 1) * 128], identity=sb_iden[:]
                )
                nc.vector.tensor_scalar(
                    out=cnt_sb[:, h * C:(h + 1) * C], in0=pt[:],
                    scalar1=1.0, scalar2=None, op0=OP.mult,
                )
            cnp_t = ptpool.tile([128, C], F32, tag="pt")
            cnp = cnp_t[0:C, :]
            for h in range(2):
                nc.tensor.matmul(
                    cnp,
                    lhsT=cnt_sb[:, h * C:(h + 1) * C],
                    rhs=cnt_sb[:, h * C:(h + 1) * C],
                    start=(h == 0),
                    stop=(h == 1),
                )
            # d2 = sq_i + sq_j - 2*CN + big*I
            d2a = tpool.tile([C, C], F32, tag="d2a")
            nc.vector.scalar_tensor_tensor(
                out=d2a[:], in0=cnp, scalar=-2.0, in1=sb_eyebig[:],
                op0=OP.mult, op1=OP.add,
            )
            d2b = tpool.tile([C, C], F32, tag="d2b")
            nc.vector.tensor_scalar(
                out=d2b[:], in0=d2a[:], scalar1=sq[:], scalar2=None, op0=OP.add
            )
            # sq as a row, broadcast down the partitions
            psr_t = ptpool.tile([128, C], F32, tag="pt")
            psr = psr_t[0:1, :]
            nc.tensor.matmul(
                psr, lhsT=sq[:], rhs=sb_iden[:],
                start=True, stop=True,
            )
            sqr_sb = tpool.tile([1, C], F32, tag="sqr_sb")
            nc.vector.tensor_scalar(
                out=sqr_sb[:], in0=psr, scalar1=1.0, scalar2=None, op0=OP.mult,
            )
            sqr_t = ptpool.tile([128, C], F32, tag="pt")
            sq_rows = sqr_t[0:C, :]
            nc.tensor.matmul(
                sq_rows, lhsT=sb_onesr[:], rhs=sqr_sb[:], start=True, stop=True
            )
            d2f = tpool.tile([C, C], F32, tag="d2f")
            nc.vector.tensor_tensor(
                out=d2f[:], in0=d2b[:], in1=sq_rows, op=OP.add
            )
            # m = sqrt(d2); rinv = 1/m  (avoids Ln/Exp act-table loads)
            mroot = tpool.tile([C, C], F32, tag="mroot")
            nc.scalar.activation(out=mroot[:], in_=d2f[:], func=AF.Sqrt)
            rinv = tpool.tile([C, C], F32, tag="rinv")
            nc.vector.reciprocal(out=rinv[:], in_=mroot[:])
            # s as a row, broadcast
            pss_t = ptpool.tile([128, C], F32, tag="pt")
            pss = pss_t[0:1, :]
            nc.tensor.matmul(
                pss, lhsT=s_sb[:], rhs=sb_iden[:],
                start=True, stop=True,
            )
            sr_sb = tpool.tile([1, C], F32, tag="sr_sb")
            nc.vector.tensor_scalar(
                out=sr_sb[:], in0=pss, scalar1=1.0, scalar2=None, op0=OP.mult,
            )
            srow_t = ptpool.tile([128, C], F32, tag="pt")
            s_rows = srow_t[0:C, :]
            nc.tensor.matmul(
                s_rows, lhsT=sb_onesr[:], rhs=sr_sb[:], start=True, stop=True
            )
            # term = wsc * (s_i + s_j) / m
            ssum = tpool.tile([C, C], F32, tag="ssum")
            nc.vector.tensor_scalar(
                out=ssum[:], in0=s_rows, scalar1=s_sb[:], scalar2=None,
                op0=OP.add,
            )
            numer = tpool.tile([C, C], F32, tag="numer")
            nc.vector.tensor_tensor(
                out=numer[:], in0=ssum[:], in1=sb_wsc[:], op=OP.mult
            )
            term = tpool.tile([C, C], F32, tag="term")
            nc.vector.tensor_tensor(
                out=term[:], in0=numer[:], in1=rinv[:], op=OP.mult
            )
            tsum = tpool.tile([C, 1], F32, tag="tsum")
            nc.vector.tensor_reduce(
                out=tsum[:], in_=term[:], axis=mybir.AxisListType.X, op=OP.add
            )
            total = tpool.tile([C, 1], F32, tag="total")
            nc.vector.scalar_tensor_tensor(
                out=total[:], in0=absr[:], scalar=1e-6, in1=tsum[:],
                op0=OP.mult, op1=OP.add,
            )
            pl_t = ptpool.tile([128, C], F32, tag="pt")
            pl = pl_t[0:1, 0:1]
            nc.tensor.matmul(
                pl, lhsT=sb_ones[:], rhs=total[:],
                start=True, stop=True,
            )
            loss_sb = tpool.tile([1, 1], F32, tag="loss_sb")
            nc.vector.tensor_scalar(
                out=loss_sb[:], in0=pl, scalar1=1.0, scalar2=None, op0=OP.mult,
            )
            nc.sync.dma_start(out=outp[:], in_=loss_sb[:])

    _split_excess_waits(nc)
    return nc


def make_host_inputs(predicted, centroids, distances, count, class_weights, target):
    pred = np.asarray(predicted, dtype=np.float32)
    cent32 = np.ascontiguousarray(np.asarray(centroids, dtype=np.float32))
    t = np.asarray(target).astype(np.int64).ravel()
    cnt = np.asarray(count, np.float64).reshape(C)
    ic = (1.0 / cnt).astype(np.float32)            # [C]
    cent_bf = cent32.astype(ml_dtypes.bfloat16)

    order = np.argsort(t, kind="stable")
    tsorted = t[order]
    starts = np.searchsorted(tsorted, np.arange(C + 1))

    aug = np.zeros((128, SS), np.float32)
    aug[0:SS, :] = np.eye(SS, dtype=np.float32)
    aug[SS, :] = -1.0
    aug2 = aug.copy()
    aug2[SS, :] = -2.0
    augs = np.concatenate([aug, aug2], axis=1)
    cn2 = np.sum(cent32.astype(np.float64) ** 2, axis=1).astype(np.float32)  # [C]

    catc = np.zeros((C, 452), np.float32)
    catc[:, 0:64] = (np.asarray(class_weights, np.float64) * (C - 1) / C)
    catc[:, 64:128] = np.eye(C) * 1e14
    catc[:, 128:192] = np.eye(C)
    catc[:, 192:448] = cent32
    catc[:, 448] = np.asarray(distances, np.float32).ravel()
    catc[:, 449] = ic
    catc[:, 450] = 1.0

    shared = dict(
        augs=augs.astype(ml_dtypes.bfloat16),
        catc=catc,
        onesr=np.ones((1, C), np.float32),
    )

    per_core = []
    for k in range(NCORES):
        idx_flat = np.full(NSUB * SS, -1, np.int64)
        sub_class = np.full(NSUB, -1, np.int64)
        pos = 0  # subtile cursor
        for c in range(k * CL, (k + 1) * CL):
            seg = order[starts[c]:starts[c + 1]]
            nst = (len(seg) + SS - 1) // SS
            assert pos + nst <= NSUB, "NSUB too small for class distribution"
            idx_flat[pos * SS:pos * SS + len(seg)] = seg
            sub_class[pos:pos + nst] = c
            pos += nst

        valid = idx_flat >= 0
        vidx = idx_flat[valid]
        xs = np.zeros((NSUB * SS, D), np.float32)
        xs[valid] = pred[vidx] * ic[t[vidx]][:, None]
        oa = np.zeros((NSUB * SS, CL), np.float32)
        oa[np.nonzero(valid)[0], (t[vidx] - k * CL)] = 1.0

        a_sub = np.zeros((NSUB, 128, W), ml_dtypes.bfloat16)
        a_sub[:, 0:SS, 0:D] = xs.reshape(NSUB, SS, D)
        a_sub[:, 0:SS, OAC:OAC + CL] = oa.reshape(NSUB, SS, CL)
        has_c = sub_class >= 0
        a_sub[has_c, SS, 0:D] = cent_bf[sub_class[has_c]]
        blk = (
            a_sub.reshape(NM, A, 128, W).transpose(0, 2, 1, 3).reshape(NSUB * 128, W)
        )
        cnsq = np.zeros((128, NSUB), np.float32)
        cnsq[:, has_c] = cn2[sub_class[has_c]][None, :]
        for s in range(NSUB):  # zero ACT-assigned subtiles: their v2 is complete
            if s % A in _act_subtiles(s // A):
                cnsq[:, s] = 0.0
        per_core.append(dict(
            blk=np.ascontiguousarray(blk), cnsq=cnsq, **shared,
        ))
    return per_core


_CACHED = {}


def run_spmd(predicted, centroids, distances, count, class_weights, target,
             trace=False, **kw):
    key = predicted.shape[0]
    if key not in _CACHED:
        _CACHED[key] = build_module()
    nc = _CACHED[key]
    in_maps = make_host_inputs(
        predicted, centroids, distances, count, class_weights, target
    )
    return run_bass_kernel_spmd(nc, in_maps, list(range(NCORES)), trace=trace, **kw)


def kernel(predicted, centroids, distances, count, class_weights, target):
    res = run_spmd(predicted, centroids, distances, count, class_weights, target)
    out = res.results[0]["out"]
    return np.asarray(out).reshape(()).astype(np.float32)
